# revision 30
# baseline (speedup 1.0000x reference)
"""Trainium2 Bass kernel for AttentionBlock (GroupNorm + cross-attn + proj + residual).

Sharding: pure data-parallel over batch. B=8 batch elements -> 8 NeuronCores,
one full batch element per core, zero collectives.

Device layout notes (per core):
  x:   (512, 1024) f32, channels on partitions (4 chunks of 128)
  Attention computed with keys-on-partitions:  ST[j,i] = sum_d k[d,j] q[d,i]
  so exp(ST) directly serves as lhsT for the O^T matmul (no transposes
  anywhere).  The softmax denominator comes from an appended ones-column in
  v^T (row 64 of the O^T accumulator).  Head pairs (2p, 2p+1) share a
  partition chunk; their ST matmuls are packed onto PE row-strips 0-63 /
  64-127 via tile_position and run concurrently into two single-head PSUM
  tiles (A/B streams) whose (128, 1024) Exps alternate back-to-back on the
  ScalarEngine -- the kernel's bottleneck stream.  All other matmul work
  (q/k/v/ctx projections, previous pair's O^T, final proj) is emitted as
  filler thunks inside the exp stream so the TensorEngine runs in the
  exp-bound gaps.
"""

import numpy as np
import ml_dtypes

import concourse.bass as bass
import concourse.mybir as mybir
import concourse.bacc as bacc
import concourse.tile as tile
from concourse.bass_utils import run_bass_kernel_spmd

B, DIM, H, W = 8, 512, 32, 32
HW = H * W
HEADS, HD, GROUPS = 8, 64, 32
CTX, L = 1024, 77
EPS = 1e-5
SCALE = HD ** -0.25
NCORES = 8
KC = 9          # key chunks: 8 spatial (128 each) + 1 ctx (77)
GSIZE = (DIM // GROUPS) * HW   # elements per group = 16*1024

BF = mybir.dt.bfloat16
F32 = mybir.dt.float32
AF = mybir.ActivationFunctionType
ALU = mybir.AluOpType
AX = mybir.AxisListType

PROFILE = False
LAST_RESULTS = None
ROW_TILE = True  # PE row-strip packing of head pairs; set False to disable

_cached_nc = None


def _build():
    nc = bacc.Bacc("TRN2", target_bir_lowering=False, debug=False,
                   num_devices=NCORES)

    xd = nc.dram_tensor("x", [DIM, HW], F32, kind="ExternalInput")
    ctxT_d = nc.dram_tensor("ctxT", [CTX, L], BF, kind="ExternalInput")
    mb_d = nc.dram_tensor("mbias", [L, 1], F32, kind="ExternalInput")
    wq_d = nc.dram_tensor("wqT", [DIM, DIM], BF, kind="ExternalInput")
    wk_d = nc.dram_tensor("wkT", [DIM, DIM], BF, kind="ExternalInput")
    wv_d = nc.dram_tensor("wvT", [DIM, DIM], BF, kind="ExternalInput")
    wp_d = nc.dram_tensor("wpT", [DIM, DIM], BF, kind="ExternalInput")
    wck_d = nc.dram_tensor("wckT", [CTX, DIM], BF, kind="ExternalInput")
    wcv_d = nc.dram_tensor("wcvT", [CTX, DIM], BF, kind="ExternalInput")
    gnw_d = nc.dram_tensor("gnw", [DIM], F32, kind="ExternalInput")
    gnb_d = nc.dram_tensor("gnb", [DIM], F32, kind="ExternalInput")
    ind_d = nc.dram_tensor("ind", [DIM, GROUPS], F32, kind="ExternalInput")
    indT_d = nc.dram_tensor("indT", [GROUPS, DIM], F32, kind="ExternalInput")
    out_d = nc.dram_tensor("out", [DIM, HW], F32, kind="ExternalOutput")

    with tile.TileContext(nc) as tc:
        with (
            tc.tile_pool(name="sb", bufs=1) as sb,
            tc.tile_pool(name="wk2", bufs=2) as wk2,
            tc.tile_pool(name="psA", bufs=1, space="PSUM") as psA,
            tc.tile_pool(name="psB", bufs=1, space="PSUM") as psB,
            tc.tile_pool(name="ps1", bufs=1, space="PSUM") as ps1,
            tc.tile_pool(name="ps3", bufs=1, space="PSUM") as ps3,
        ):
            # ---------- DMA inputs: x + GN + q/k weights first (head
            # critical path), ctx/v/proj weights stream in behind
            x_sb = sb.tile([128, 4, HW], F32, tag="x")
            xre = xd.ap().rearrange("(a p) m -> p a m", p=128)
            for c in range(4):
                (nc.sync if c % 2 == 0 else nc.scalar).dma_start(
                    x_sb[:, c, :], xre[:, c, :])
            gnw = sb.tile([128, 4], F32, tag="gnw")
            nc.sync.dma_start(gnw[:], gnw_d.ap().rearrange("(a p) -> p a", p=128))
            gnb = sb.tile([128, 4], F32, tag="gnb")
            nc.sync.dma_start(gnb[:], gnb_d.ap().rearrange("(a p) -> p a", p=128))
            ind = sb.tile([128, 4, GROUPS], F32, tag="ind")
            nc.sync.dma_start(ind[:], ind_d.ap().rearrange("(a p) g -> p a g", p=128))
            indT = sb.tile([GROUPS, 4, 128], F32, tag="indT")
            nc.sync.dma_start(indT[:], indT_d.ap().rearrange("g (a c) -> g a c", c=128))
            wq = sb.tile([128, 4, DIM], BF, tag="wq")
            nc.sync.dma_start(wq[:], wq_d.ap().rearrange("(a p) o -> p a o", p=128))
            wk = sb.tile([128, 4, DIM], BF, tag="wk")
            nc.scalar.dma_start(wk[:], wk_d.ap().rearrange("(a p) o -> p a o", p=128))
            mb = sb.tile([L, 1], F32, tag="mb")
            nc.sync.dma_start(mb[:], mb_d.ap())
            ctxT = sb.tile([128, 8, L], BF, tag="ctxT")
            nc.sync.dma_start(ctxT[:], ctxT_d.ap().rearrange("(a p) l -> p a l", p=128))
            wck = sb.tile([128, 8, DIM], BF, tag="wck")
            nc.scalar.dma_start(wck[:], wck_d.ap().rearrange("(a p) o -> p a o", p=128))
            wv = sb.tile([128, 4, DIM], BF, tag="wv")
            nc.sync.dma_start(wv[:], wv_d.ap().rearrange("(a p) o -> p a o", p=128))
            wcv = sb.tile([128, 8, DIM], BF, tag="wcv")
            nc.scalar.dma_start(wcv[:], wcv_d.ap().rearrange("(a p) o -> p a o", p=128))
            wp = sb.tile([128, 4, DIM], BF, tag="wp")
            nc.scalar.dma_start(wp[:], wp_d.ap().rearrange("(a p) o -> p a o", p=128))

            # PE warm-up: dense matmul burst during the input-DMA head so
            # HAM reaches 2.4 GHz before the real stream begins
            warm = sb.tile([128, 256], BF, tag="warm")
            nc.vector.memset(warm[:], 0.0)
            wps = ps1.tile([128, 256], F32, tag="st", name="warmps")
            for i in range(48):
                nc.tensor.matmul(wps[:], warm[:, 0:128], warm[:, 0:256],
                                 start=True, stop=True)

            q = sb.tile([128, 4, HW], BF, tag="q")
            k = sb.tile([128, 4, HW], BF, tag="k")
            ck = sb.tile([128, 4, L], BF, tag="ck")
            vT = sb.tile([128, KC, HEADS, HD + 1], BF, tag="vT")
            Ofull = sb.tile([128, 4, HW], BF, tag="Ofull")

            def ck_thunk(p):
                def t():
                    ps = ps1.tile([128, HW], F32, tag="st", name=f"ck{p}")
                    for kc in range(8):
                        nc.tensor.matmul(ps[:, 0:L], wck[:, kc, p*128:(p+1)*128],
                                         ctxT[:, kc, :], start=(kc == 0),
                                         stop=(kc == 7))
                    nc.vector.tensor_copy(ck[:, p, :], ps[:, 0:L])
                return t

            def cv_thunk():
                def t():
                    psv = ps3.tile([L, 512], F32, tag="ot", name="cv")
                    for kc in range(8):
                        nc.tensor.matmul(psv[:], ctxT[:, kc, :], wcv[:, kc, :],
                                         start=(kc == 0), stop=(kc == 7))
                    nc.vector.tensor_copy(
                        vT[0:L, 8, :, 0:HD],
                        psv[:].rearrange("p (h d) -> p h d", h=HEADS))
                return t

            def qk_thunks(p):
                """q/k projections for head-pair chunk p (2 thunks)."""
                thunks = []
                for wt, dst in ((wq, q), (wk, k)):
                    def one(wt=wt, dst=dst):
                        ps = ps1.tile([128, HW], F32, tag="st",
                                      name=f"qk{p}_{dst.name}")
                        for n in range(2):
                            for kc in range(4):
                                nc.tensor.matmul(ps[:, n*512:(n+1)*512],
                                                 wt[:, kc, p*128:(p+1)*128],
                                                 xn[:, kc, n*512:(n+1)*512],
                                                 start=(kc == 0), stop=(kc == 3))
                        nc.vector.tensor_copy(dst[:, p, :], ps[:])
                    thunks.append(one)
                return thunks

            def qk_proj(p):
                for t in qk_thunks(p):
                    t()

            E_tiles = {}

            def spread(thunks, m_lo=0, m_hi=KC - 1):
                """distribute a flat thunk list over m slots [m_lo, m_hi]."""
                fb = [[] for _ in range(KC)]
                nm = m_hi - m_lo + 1
                per = (len(thunks) + nm - 1) // nm if thunks else 0
                i = 0
                for m in range(m_lo, m_hi + 1):
                    for _ in range(per):
                        if i < len(thunks):
                            fb[m].append(thunks[i])
                            i += 1
                while i < len(thunks):
                    fb[m_hi].append(thunks[i])
                    i += 1
                return fb

            def merge(*fbs):
                out = [[] for _ in range(KC)]
                for fb in fbs:
                    for m in range(KC):
                        out[m].extend(fb[m])
                return out

            def st_pair(p, fillers_by_m=None):
                fb = fillers_by_m or [[] for _ in range(KC)]
                E = wk2.tile([128, KC, 2, HW], BF, tag="E", name=f"E{p}")
                E_tiles[p] = E
                def st_mms(m, h2, pool, tg):
                    rows = 128 if m < 8 else L
                    base = h2 * 64
                    pp = pool.tile([128, HW], F32, tag=tg,
                                   name=f"pp{p}_{m}_{h2}")
                    for n in range(2):
                        if m < 8:
                            lhsT = k[base:base+64, p, m*128:(m+1)*128]
                        else:
                            lhsT = ck[base:base+64, p, :]
                        rhs = q[base:base+64, p, n*512:(n+1)*512]
                        tp = (base, 0) if ROW_TILE else None
                        nc.tensor.matmul(pp[0:rows, n*512:(n+1)*512],
                                         lhsT, rhs, start=True, stop=True,
                                         tile_position=tp)
                    return pp

                def st_exp(m, h2, pp):
                    rows = 128 if m < 8 else L
                    dstE = E[0:rows, m, h2, :]
                    if m == 8:
                        nc.scalar.activation(dstE, pp[0:rows, :], AF.Exp,
                                             bias=mb[:])
                    else:
                        nc.scalar.activation(dstE, pp[0:rows, :], AF.Exp)

                # software-pipelined: A-stream matmuls run one m ahead, so
                # exp(A, m+1) is ready the moment exp(B, m) retires
                ppA = st_mms(0, 0, psA, "stA")
                for m in range(KC):
                    st_exp(m, 0, ppA)
                    ppB = st_mms(m, 1, psB, "stB")
                    if m < KC - 1:
                        ppA = st_mms(m + 1, 0, psA, "stA")
                    st_exp(m, 1, ppB)
                    for t in fb[m]:
                        t()

            def ot_head_thunks(p, h2, pool_tag):
                """thunks for one head's O^T: [alloc, n0:kc0-2, n0:kc3-5,
                n0:kc6-8, norm_n0, n1:kc0-2, n1:kc3-5, n1:kc6-8, norm_n1];
                norm of each n-half overlaps the other half's matmuls."""
                h = 2 * p + h2
                base = h2 * 64
                pool, tag = pool_tag
                state = {}

                def alloc():
                    state["ot"] = pool.tile([HD + 1, HW], F32, tag=tag,
                                            name=f"ot{h}")
                    state["rbs"] = wk2.tile([64, HW], F32, tag="rbs",
                                            name=f"rbs{h}")
                thunks = [alloc]
                for n in range(2):
                    for kc0 in range(0, KC, 3):
                        def mms(n=n, kc0=kc0):
                            E = E_tiles[p]
                            otps = state["ot"]
                            for kc in range(kc0, min(kc0 + 3, KC)):
                                rows = 128 if kc < 8 else L
                                nc.tensor.matmul(
                                    otps[:, n*512:(n+1)*512],
                                    vT[0:rows, kc, h, :],
                                    E[0:rows, kc, h2, n*512:(n+1)*512],
                                    start=(kc == 0), stop=(kc == KC - 1))
                        thunks.append(mms)

                    def norm(n=n):
                        otps = state["ot"]
                        ns = slice(n*512, (n+1)*512)
                        rsb = wk2.tile([1, 512], F32, tag="rsb",
                                       name=f"rsb{h}_{n}")
                        nc.vector.reciprocal(rsb[:], otps[64:65, ns])
                        rbs = state["rbs"]
                        nc.gpsimd.partition_broadcast(rbs[:, ns], rsb[:])
                        nc.vector.tensor_tensor(Ofull[base:base+64, p, ns],
                                                otps[0:64, ns], rbs[:, ns],
                                                op=ALU.mult)
                    thunks.append(norm)
                return thunks

            def ot_thunks(p):
                return (ot_head_thunks(p, 0, (ps3, "ot"))
                        + ot_head_thunks(p, 1, (ps3, "ot")))

            # interleaved schedule: pair 0's projections, its attention, then
            # vT + later pairs fill PE gaps while ACT streams the exps
            # program order defines both the dependency graph and each
            # engine's in-order instruction stream: ctx projections go first
            # (their DMAs land earliest, PE warms up during GroupNorm), then
            # GN, then the attention pairs; the scheduler fills PE gaps in
            # the ACT-bound exp stream with vt/qk of the next pair.
            nc.vector.memset(vT[:, :, :, HD:HD+1], 1.0)
            # ---------- GroupNorm stats ----------
            stats = sb.tile([128, 4, 2], F32, tag="stats")
            for c in range(4):
                scr = wk2.tile([128, HW], BF, tag="sq", name=f"scr{c}")
                nc.scalar.activation(scr[:], x_sb[:, c, :], AF.Identity,
                                     accum_out=stats[:, c, 0:1])
                sq = wk2.tile([128, HW], BF, tag="sq", name=f"sq{c}")
                nc.scalar.activation(sq[:], x_sb[:, c, :], AF.Square,
                                     accum_out=stats[:, c, 1:2])
            st_ps = ps3.tile([GROUPS, 2], F32, tag="ot")
            for c in range(4):
                nc.tensor.matmul(st_ps[:], ind[:, c, :], stats[:, c, :],
                                 start=(c == 0), stop=(c == 3))
            sg = sb.tile([GROUPS, 2], F32, tag="sg")
            nc.vector.tensor_scalar_mul(sg[:], st_ps[:], 1.0 / GSIZE)
            var = sb.tile([GROUPS, 1], F32, tag="var")
            nc.vector.scalar_tensor_tensor(var[:], sg[:, 0:1], -1.0, sg[:, 0:1],
                                           op0=ALU.mult, op1=ALU.mult)
            nc.vector.tensor_add(var[:], var[:], sg[:, 1:2])
            nc.vector.tensor_scalar_add(var[:], var[:], EPS)
            sqv = sb.tile([GROUPS, 1], F32, tag="sqv")
            nc.scalar.activation(sqv[:], var[:], AF.Sqrt)
            gst = sb.tile([GROUPS, 2], F32, tag="gst")
            nc.vector.tensor_copy(gst[:, 0:1], sg[:, 0:1])
            nc.vector.reciprocal(gst[:, 1:2], sqv[:])

            xn = sb.tile([128, 4, HW], BF, tag="xn")
            scale_t = sb.tile([128, 4], F32, tag="scale")
            bias_t = sb.tile([128, 4], F32, tag="bias")
            for c in range(4):
                cb = ps3.tile([128, 2], F32, tag="ot")
                nc.tensor.matmul(cb[:], indT[:, c, :], gst[:], start=True, stop=True)
                nc.vector.tensor_mul(scale_t[:, c:c+1], gnw[:, c:c+1], cb[:, 1:2])
                nc.vector.scalar_tensor_tensor(bias_t[:, c:c+1], cb[:, 0:1], -1.0,
                                               scale_t[:, c:c+1],
                                               op0=ALU.mult, op1=ALU.mult)
                nc.vector.tensor_add(bias_t[:, c:c+1], bias_t[:, c:c+1], gnb[:, c:c+1])
                nc.scalar.activation(xn[:, c, :], x_sb[:, c, :], AF.Identity,
                                     bias=bias_t[:, c:c+1],
                                     scale=scale_t[:, c:c+1])


            def vt_thunks():
                thunks = []
                for m in range(8):
                    def sp(m=m):
                        ps = ps1.tile([128, HW], F32, tag="st", name=f"vt{m}")
                        for kc in range(4):
                            nc.tensor.matmul(ps[:, 0:512],
                                             xn[:, kc, m*128:(m+1)*128],
                                             wv[:, kc, :],
                                             start=(kc == 0), stop=(kc == 3))
                        nc.vector.tensor_copy(
                            vT[:, m, :, 0:HD],
                            ps[:, 0:512].rearrange("p (h d) -> p h d", h=HEADS))
                    thunks.append(sp)
                return thunks

            def at_m(thunks, *ms):
                fb = [[] for _ in range(KC)]
                for t, m in zip(thunks, ms):
                    fb[m].append(t)
                return fb

            qk_proj(0)
            st_pair(0, merge(at_m([ck_thunk(0), cv_thunk(), ck_thunk(1)],
                                  0, 1, 5),
                             spread(vt_thunks(), 0, 8),
                             at_m(qk_thunks(1), 3, 4)))
            st_pair(1, merge(spread(ot_thunks(0), 0, 4),
                             at_m(qk_thunks(2), 5, 6),
                             at_m([ck_thunk(2)], 7)))
            st_pair(2, merge(spread(ot_thunks(1), 0, 4),
                             at_m(qk_thunks(3), 5, 6),
                             at_m([ck_thunk(3)], 7)))
            # pair 3: drain OT(2) early (m 0-4), then pipeline BOTH heads'
            # O^T behind the exp stream -- each kc group only needs the exps
            # emitted so far.  OT(3A) uses the ps3 slot, OT(3B) the ps1 slot.
            fb3 = [[] for _ in range(KC)]
            for th, (pool, tg) in ((0, (ps3, "ot")), (1, (ps1, "st"))):
                t3 = ot_head_thunks(3, th, (pool, tg))
                # t3 = [alloc, n0kc02, n0kc35, n0kc68, norm0,
                #       n1kc02, n1kc35, n1kc68, norm1]
                fb3[5] += [t3[0], t3[1], t3[5]]          # alloc, kc0-2 both n
                fb3[6] += [t3[2], t3[6]]                 # kc3-5 both n
                fb3[8] += [t3[3], t3[4], t3[7], t3[8]]   # kc6-8 + norms
            st_pair(3, merge(spread(ot_thunks(2), 0, 4), fb3))

            # ---------- proj + residual ----------
            outsb = sb.tile([128, 4, HW], F32, tag="outsb")
            proj_pools = [(ps3, "ot"), (ps1, "st"), (psA, "stA"), (psB, "stB")]
            for m in range(4):
                pool_m, tag_m = proj_pools[m]
                ps = pool_m.tile([128, HW], F32, tag=tag_m, name=f"proj{m}")
                for n in range(2):
                    for kc in range(4):
                        nc.tensor.matmul(ps[:, n*512:(n+1)*512],
                                         wp[:, kc, m*128:(m+1)*128],
                                         Ofull[:, kc, n*512:(n+1)*512],
                                         start=(kc == 0), stop=(kc == 3))
                nc.vector.scalar_tensor_tensor(outsb[:, m, :], ps[:], 1.0,
                                               x_sb[:, m, :],
                                               op0=ALU.mult, op1=ALU.add)
                nc.sync.dma_start(
                    out_d.ap().rearrange("(a p) m -> p a m", p=128)[:, m, :],
                    outsb[:, m, :])

    nc.compile()
    return nc


def _get_nc():
    global _cached_nc
    if _cached_nc is None:
        _cached_nc = _build()
    return _cached_nc


def kernel(**inputs):
    global LAST_RESULTS
    x = np.ascontiguousarray(np.asarray(inputs["x"], dtype=np.float32))
    context = np.asarray(inputs["context"], dtype=np.float32)
    mask = np.asarray(inputs["mask"])
    gn_w = np.asarray(inputs["gn_w"], dtype=np.float32)
    gn_b = np.asarray(inputs["gn_b"], dtype=np.float32)
    qkv_w = np.asarray(inputs["qkv_w"], dtype=np.float32)
    ckv_w = np.asarray(inputs["ckv_w"], dtype=np.float32)
    proj_w = np.asarray(inputs["proj_w"], dtype=np.float32)
    # biases are zero in this problem's setup; fold-capable but unused
    bf = ml_dtypes.bfloat16

    wqT = np.ascontiguousarray((qkv_w[0:DIM].T * SCALE).astype(bf))
    wkT = np.ascontiguousarray((qkv_w[DIM:2*DIM].T * SCALE).astype(bf))
    wvT = np.ascontiguousarray(qkv_w[2*DIM:3*DIM].T.astype(bf))
    wpT = np.ascontiguousarray(proj_w.T.astype(bf))
    wckT = np.ascontiguousarray((ckv_w[0:DIM].T * SCALE).astype(bf))
    wcvT = np.ascontiguousarray(ckv_w[DIM:2*DIM].T.astype(bf))
    gidx = np.arange(DIM) // (DIM // GROUPS)
    ind = (gidx[:, None] == np.arange(GROUPS)[None, :]).astype(np.float32)
    indT = np.ascontiguousarray(ind.T)

    shared = {"wqT": wqT, "wkT": wkT, "wvT": wvT, "wpT": wpT,
              "wckT": wckT, "wcvT": wcvT,
              "gnw": gn_w, "gnb": gn_b, "ind": ind, "indT": indT}
    in_maps = []
    for b in range(B):
        mbias = ((mask[b].astype(np.float32) - 1.0) * 30000.0).reshape(L, 1)
        im = dict(shared)
        im["x"] = x[b].reshape(DIM, HW)
        im["ctxT"] = np.ascontiguousarray(context[b].T.astype(bf))
        im["mbias"] = mbias
        in_maps.append(im)

    nc = _get_nc()
    res = run_bass_kernel_spmd(nc, in_maps, core_ids=list(range(NCORES)),
                               trace=PROFILE)
    LAST_RESULTS = res
    out = np.stack([res.results[b]["out"].reshape(DIM, H, W) for b in range(B)])
    return out.astype(np.float32)


# revision 32
# speedup vs baseline: 1.3676x; 1.3676x over previous
"""Trainium2 Bass kernel for AttentionBlock (GroupNorm + cross-attn + proj + residual).

Sharding: pure data-parallel over batch. B=8 batch elements -> 8 NeuronCores,
one full batch element per core, zero collectives.

Device layout notes (per core):
  x:   (512, 1024) f32, channels on partitions (4 chunks of 128)
  Attention computed with keys-on-partitions:  ST[j,i] = sum_d k[d,j] q[d,i]
  so exp(ST) directly serves as lhsT for the O^T matmul (no transposes
  anywhere).  The softmax denominator comes from an appended ones-column in
  v^T (row 64 of the O^T accumulator).  Head pairs (2p, 2p+1) share a
  partition chunk; their ST matmuls are packed onto PE row-strips 0-63 /
  64-127 via tile_position and run concurrently into two single-head PSUM
  tiles (A/B streams) whose (128, 1024) Exps alternate back-to-back on the
  ScalarEngine -- the kernel's bottleneck stream.  All other matmul work
  (q/k/v/ctx projections, previous pair's O^T, final proj) is emitted as
  filler thunks inside the exp stream so the TensorEngine runs in the
  exp-bound gaps.
"""

import numpy as np
import ml_dtypes

import concourse.bass as bass
import concourse.mybir as mybir
import concourse.bacc as bacc
import concourse.tile as tile
from concourse.bass_utils import run_bass_kernel_spmd

B, DIM, H, W = 8, 512, 32, 32
HW = H * W
HEADS, HD, GROUPS = 8, 64, 32
CTX, L = 1024, 77
EPS = 1e-5
SCALE = HD ** -0.25
NCORES = 8
KC = 9          # key chunks: 8 spatial (128 each) + 1 ctx (77)
GSIZE = (DIM // GROUPS) * HW   # elements per group = 16*1024

BF = mybir.dt.bfloat16
F32 = mybir.dt.float32
AF = mybir.ActivationFunctionType
ALU = mybir.AluOpType
AX = mybir.AxisListType

PROFILE = False
LAST_RESULTS = None
ROW_TILE = True  # PE row-strip packing of head pairs; set False to disable

_cached_nc = None


def _build():
    nc = bacc.Bacc("TRN2", target_bir_lowering=False, debug=False,
                   num_devices=NCORES)

    xd = nc.dram_tensor("x", [DIM, HW], F32, kind="ExternalInput")
    ctxT_d = nc.dram_tensor("ctxT", [CTX, L], BF, kind="ExternalInput")
    mb_d = nc.dram_tensor("mbias", [L, 1], F32, kind="ExternalInput")
    wq_d = nc.dram_tensor("wqT", [DIM, DIM], BF, kind="ExternalInput")
    wk_d = nc.dram_tensor("wkT", [DIM, DIM], BF, kind="ExternalInput")
    wv_d = nc.dram_tensor("wvT", [DIM, DIM], BF, kind="ExternalInput")
    wp_d = nc.dram_tensor("wpT", [DIM, DIM], BF, kind="ExternalInput")
    wck_d = nc.dram_tensor("wckT", [CTX, DIM], BF, kind="ExternalInput")
    wcv_d = nc.dram_tensor("wcvT", [CTX, DIM], BF, kind="ExternalInput")
    gnw_d = nc.dram_tensor("gnw", [DIM], F32, kind="ExternalInput")
    gnb_d = nc.dram_tensor("gnb", [DIM], F32, kind="ExternalInput")
    ind_d = nc.dram_tensor("ind", [DIM, GROUPS], F32, kind="ExternalInput")
    indT_d = nc.dram_tensor("indT", [GROUPS, DIM], F32, kind="ExternalInput")
    out_d = nc.dram_tensor("out", [DIM, HW], F32, kind="ExternalOutput")

    with tile.TileContext(nc) as tc:
        with (
            tc.tile_pool(name="sb", bufs=1) as sb,
            tc.tile_pool(name="wk2", bufs=2) as wk2,
            tc.tile_pool(name="psA", bufs=1, space="PSUM") as psA,
            tc.tile_pool(name="psB", bufs=1, space="PSUM") as psB,
            tc.tile_pool(name="ps1", bufs=1, space="PSUM") as ps1,
            tc.tile_pool(name="ps3", bufs=1, space="PSUM") as ps3,
        ):
            # ---------- DMA inputs: x + GN + q/k weights first (head
            # critical path), ctx/v/proj weights stream in behind
            x_sb = sb.tile([128, 4, HW], F32, tag="x")
            xre = xd.ap().rearrange("(a p) m -> p a m", p=128)
            for c in range(4):
                (nc.sync if c % 2 == 0 else nc.scalar).dma_start(
                    x_sb[:, c, :], xre[:, c, :])
            gnw = sb.tile([128, 4], F32, tag="gnw")
            nc.sync.dma_start(gnw[:], gnw_d.ap().rearrange("(a p) -> p a", p=128))
            gnb = sb.tile([128, 4], F32, tag="gnb")
            nc.sync.dma_start(gnb[:], gnb_d.ap().rearrange("(a p) -> p a", p=128))
            ind = sb.tile([128, 4, GROUPS], F32, tag="ind")
            nc.sync.dma_start(ind[:], ind_d.ap().rearrange("(a p) g -> p a g", p=128))
            indT = sb.tile([GROUPS, 4, 128], F32, tag="indT")
            nc.sync.dma_start(indT[:], indT_d.ap().rearrange("g (a c) -> g a c", c=128))
            wq = sb.tile([128, 4, DIM], BF, tag="wq")
            nc.sync.dma_start(wq[:], wq_d.ap().rearrange("(a p) o -> p a o", p=128))
            wk = sb.tile([128, 4, DIM], BF, tag="wk")
            nc.scalar.dma_start(wk[:], wk_d.ap().rearrange("(a p) o -> p a o", p=128))
            mb = sb.tile([L, 1], F32, tag="mb")
            nc.sync.dma_start(mb[:], mb_d.ap())
            ctxT = sb.tile([128, 8, L], BF, tag="ctxT")
            nc.sync.dma_start(ctxT[:], ctxT_d.ap().rearrange("(a p) l -> p a l", p=128))
            wck = sb.tile([128, 8, DIM], BF, tag="wck")
            nc.scalar.dma_start(wck[:], wck_d.ap().rearrange("(a p) o -> p a o", p=128))
            wv = sb.tile([128, 4, DIM], BF, tag="wv")
            nc.sync.dma_start(wv[:], wv_d.ap().rearrange("(a p) o -> p a o", p=128))
            wcv = sb.tile([128, 8, DIM], BF, tag="wcv")
            nc.scalar.dma_start(wcv[:], wcv_d.ap().rearrange("(a p) o -> p a o", p=128))
            wp = sb.tile([128, 4, DIM], BF, tag="wp")
            nc.scalar.dma_start(wp[:], wp_d.ap().rearrange("(a p) o -> p a o", p=128))

            # PE warm-up: dense matmul burst during the input-DMA head so
            # HAM reaches 2.4 GHz before the real stream begins
            warm = sb.tile([128, 256], BF, tag="warm")
            nc.vector.memset(warm[:], 0.0)
            wps = ps1.tile([128, 256], F32, tag="st", name="warmps")
            for i in range(48):
                nc.tensor.matmul(wps[:], warm[:, 0:128], warm[:, 0:256],
                                 start=True, stop=True)

            q = sb.tile([128, 4, HW], BF, tag="q")
            k = sb.tile([128, 4, HW], BF, tag="k")
            ck = sb.tile([128, 4, L], BF, tag="ck")
            vT = sb.tile([128, KC, HEADS, HD + 1], BF, tag="vT")
            Ofull = sb.tile([128, 4, HW], BF, tag="Ofull")

            def ck_thunk(p):
                def t():
                    ps = ps1.tile([128, HW], F32, tag="st", name=f"ck{p}")
                    for kc in range(8):
                        nc.tensor.matmul(ps[:, 0:L], wck[:, kc, p*128:(p+1)*128],
                                         ctxT[:, kc, :], start=(kc == 0),
                                         stop=(kc == 7))
                    nc.vector.tensor_copy(ck[:, p, :], ps[:, 0:L])
                return t

            def cv_thunk():
                def t():
                    psv = ps3.tile([L, 512], F32, tag="ot", name="cv")
                    for kc in range(8):
                        nc.tensor.matmul(psv[:], ctxT[:, kc, :], wcv[:, kc, :],
                                         start=(kc == 0), stop=(kc == 7))
                    nc.vector.tensor_copy(
                        vT[0:L, 8, :, 0:HD],
                        psv[:].rearrange("p (h d) -> p h d", h=HEADS))
                return t

            def qk_thunks(p):
                """q/k projections for head-pair chunk p (2 thunks)."""
                thunks = []
                for wt, dst in ((wq, q), (wk, k)):
                    def one(wt=wt, dst=dst):
                        ps = ps1.tile([128, HW], F32, tag="st",
                                      name=f"qk{p}_{dst.name}")
                        for n in range(2):
                            for kc in range(4):
                                nc.tensor.matmul(ps[:, n*512:(n+1)*512],
                                                 wt[:, kc, p*128:(p+1)*128],
                                                 xn[:, kc, n*512:(n+1)*512],
                                                 start=(kc == 0), stop=(kc == 3))
                        nc.vector.tensor_copy(dst[:, p, :], ps[:])
                    thunks.append(one)
                return thunks

            def qk_proj(p):
                for t in qk_thunks(p):
                    t()

            E_tiles = {}

            def spread(thunks, m_lo=0, m_hi=KC - 1):
                """distribute a flat thunk list over m slots [m_lo, m_hi]."""
                fb = [[] for _ in range(KC)]
                nm = m_hi - m_lo + 1
                per = (len(thunks) + nm - 1) // nm if thunks else 0
                i = 0
                for m in range(m_lo, m_hi + 1):
                    for _ in range(per):
                        if i < len(thunks):
                            fb[m].append(thunks[i])
                            i += 1
                while i < len(thunks):
                    fb[m_hi].append(thunks[i])
                    i += 1
                return fb

            def merge(*fbs):
                out = [[] for _ in range(KC)]
                for fb in fbs:
                    for m in range(KC):
                        out[m].extend(fb[m])
                return out

            def st_pair(p, fillers_by_m=None):
                fb = fillers_by_m or [[] for _ in range(KC)]
                E = wk2.tile([128, KC, 2, HW], BF, tag="E", name=f"E{p}")
                E_tiles[p] = E
                def st_mms(m, h2, pool, tg):
                    rows = 128 if m < 8 else L
                    base = h2 * 64
                    pp = pool.tile([128, HW], F32, tag=tg,
                                   name=f"pp{p}_{m}_{h2}")
                    for n in range(2):
                        if m < 8:
                            lhsT = k[base:base+64, p, m*128:(m+1)*128]
                        else:
                            lhsT = ck[base:base+64, p, :]
                        rhs = q[base:base+64, p, n*512:(n+1)*512]
                        tp = (base, 0) if ROW_TILE else None
                        nc.tensor.matmul(pp[0:rows, n*512:(n+1)*512],
                                         lhsT, rhs, start=True, stop=True,
                                         tile_position=tp)
                    return pp

                def st_exp(m, h2, pp):
                    rows = 128 if m < 8 else L
                    dstE = E[0:rows, m, h2, :]
                    if m == 8:
                        nc.scalar.activation(dstE, pp[0:rows, :], AF.Exp,
                                             bias=mb[:])
                    else:
                        nc.scalar.activation(dstE, pp[0:rows, :], AF.Exp)

                # software-pipelined: A-stream matmuls run one m ahead, so
                # exp(A, m+1) is ready the moment exp(B, m) retires
                ppA = st_mms(0, 0, psA, "stA")
                for m in range(KC):
                    st_exp(m, 0, ppA)
                    ppB = st_mms(m, 1, psB, "stB")
                    if m < KC - 1:
                        ppA = st_mms(m + 1, 0, psA, "stA")
                    st_exp(m, 1, ppB)
                    for t in fb[m]:
                        t()

            def ot_head_thunks(p, h2, pool_tag):
                """thunks for one head's O^T: [alloc, n0:kc0-2, n0:kc3-5,
                n0:kc6-8, norm_n0, n1:kc0-2, n1:kc3-5, n1:kc6-8, norm_n1];
                norm of each n-half overlaps the other half's matmuls."""
                h = 2 * p + h2
                base = h2 * 64
                pool, tag = pool_tag
                state = {}

                def alloc():
                    state["ot"] = pool.tile([HD + 1, HW], F32, tag=tag,
                                            name=f"ot{h}")
                    state["rbs"] = wk2.tile([64, HW], F32, tag="rbs",
                                            name=f"rbs{h}")
                thunks = [alloc]
                for n in range(2):
                    for kc0 in range(0, KC, 3):
                        def mms(n=n, kc0=kc0):
                            E = E_tiles[p]
                            otps = state["ot"]
                            for kc in range(kc0, min(kc0 + 3, KC)):
                                rows = 128 if kc < 8 else L
                                nc.tensor.matmul(
                                    otps[:, n*512:(n+1)*512],
                                    vT[0:rows, kc, h, :],
                                    E[0:rows, kc, h2, n*512:(n+1)*512],
                                    start=(kc == 0), stop=(kc == KC - 1))
                        thunks.append(mms)

                    def norm(n=n):
                        otps = state["ot"]
                        ns = slice(n*512, (n+1)*512)
                        rsb = wk2.tile([1, 512], F32, tag="rsq",
                                       name=f"rsb{h}_{n}")
                        nc.vector.reciprocal_approx_fast(rsb[:], otps[64:65, ns])
                        rbs = state["rbs"]
                        nc.gpsimd.partition_broadcast(rbs[:, ns], rsb[:])
                        nc.vector.tensor_tensor(Ofull[base:base+64, p, ns],
                                                otps[0:64, ns], rbs[:, ns],
                                                op=ALU.mult)
                    thunks.append(norm)
                return thunks

            def ot_thunks(p):
                a = ot_head_thunks(p, 0, (ps3, "ot"))
                b = ot_head_thunks(p, 1, (ps1, "st"))
                out = []
                for i in range(max(len(a), len(b))):
                    if i < len(a):
                        out.append(a[i])
                    if i < len(b):
                        out.append(b[i])
                return out

            # interleaved schedule: pair 0's projections, its attention, then
            # vT + later pairs fill PE gaps while ACT streams the exps
            # program order defines both the dependency graph and each
            # engine's in-order instruction stream: ctx projections go first
            # (their DMAs land earliest, PE warms up during GroupNorm), then
            # GN, then the attention pairs; the scheduler fills PE gaps in
            # the ACT-bound exp stream with vt/qk of the next pair.
            nc.vector.memset(vT[:, :, :, HD:HD+1], 1.0)
            # ---------- GroupNorm stats ----------
            stats = sb.tile([128, 4, 2], F32, tag="stats")
            for c in range(4):
                scr = wk2.tile([128, HW], BF, tag="sq", name=f"scr{c}")
                nc.scalar.activation(scr[:], x_sb[:, c, :], AF.Identity,
                                     accum_out=stats[:, c, 0:1])
                sq = wk2.tile([128, HW], BF, tag="sq", name=f"sq{c}")
                nc.scalar.activation(sq[:], x_sb[:, c, :], AF.Square,
                                     accum_out=stats[:, c, 1:2])
            st_ps = ps3.tile([GROUPS, 2], F32, tag="ot")
            for c in range(4):
                nc.tensor.matmul(st_ps[:], ind[:, c, :], stats[:, c, :],
                                 start=(c == 0), stop=(c == 3))
            sg = sb.tile([GROUPS, 2], F32, tag="sg")
            nc.vector.tensor_scalar_mul(sg[:], st_ps[:], 1.0 / GSIZE)
            var = sb.tile([GROUPS, 1], F32, tag="var")
            nc.vector.scalar_tensor_tensor(var[:], sg[:, 0:1], -1.0, sg[:, 0:1],
                                           op0=ALU.mult, op1=ALU.mult)
            nc.vector.tensor_add(var[:], var[:], sg[:, 1:2])
            nc.vector.tensor_scalar_add(var[:], var[:], EPS)
            sqv = sb.tile([GROUPS, 1], F32, tag="sqv")
            nc.scalar.activation(sqv[:], var[:], AF.Sqrt)
            gst = sb.tile([GROUPS, 2], F32, tag="gst")
            nc.vector.tensor_copy(gst[:, 0:1], sg[:, 0:1])
            nc.vector.reciprocal(gst[:, 1:2], sqv[:])

            xn = sb.tile([128, 4, HW], BF, tag="xn")
            scale_t = sb.tile([128, 4], F32, tag="scale")
            bias_t = sb.tile([128, 4], F32, tag="bias")
            for c in range(4):
                cb = ps3.tile([128, 2], F32, tag="ot")
                nc.tensor.matmul(cb[:], indT[:, c, :], gst[:], start=True, stop=True)
                nc.vector.tensor_mul(scale_t[:, c:c+1], gnw[:, c:c+1], cb[:, 1:2])
                nc.vector.scalar_tensor_tensor(bias_t[:, c:c+1], cb[:, 0:1], -1.0,
                                               scale_t[:, c:c+1],
                                               op0=ALU.mult, op1=ALU.mult)
                nc.vector.tensor_add(bias_t[:, c:c+1], bias_t[:, c:c+1], gnb[:, c:c+1])
                nc.scalar.activation(xn[:, c, :], x_sb[:, c, :], AF.Identity,
                                     bias=bias_t[:, c:c+1],
                                     scale=scale_t[:, c:c+1])


            def vt_thunks():
                thunks = []
                for m in range(8):
                    def sp(m=m):
                        ps = ps1.tile([128, HW], F32, tag="st", name=f"vt{m}")
                        for kc in range(4):
                            nc.tensor.matmul(ps[:, 0:512],
                                             xn[:, kc, m*128:(m+1)*128],
                                             wv[:, kc, :],
                                             start=(kc == 0), stop=(kc == 3))
                        nc.vector.tensor_copy(
                            vT[:, m, :, 0:HD],
                            ps[:, 0:512].rearrange("p (h d) -> p h d", h=HEADS))
                    thunks.append(sp)
                return thunks

            def at_m(thunks, *ms):
                fb = [[] for _ in range(KC)]
                for t, m in zip(thunks, ms):
                    fb[m].append(t)
                return fb

            qk_proj(0)
            st_pair(0, merge(at_m([ck_thunk(0), cv_thunk(), ck_thunk(1)],
                                  0, 1, 5),
                             spread(vt_thunks(), 0, 8),
                             at_m(qk_thunks(1), 3, 4)))
            st_pair(1, merge(spread(ot_thunks(0), 0, 4),
                             at_m(qk_thunks(2), 5, 6),
                             at_m([ck_thunk(2)], 7)))
            st_pair(2, merge(spread(ot_thunks(1), 0, 4),
                             at_m(qk_thunks(3), 5, 6),
                             at_m([ck_thunk(3)], 7)))
            # pair 3: drain OT(2) early (m 0-4), then pipeline BOTH heads'
            # O^T behind the exp stream -- each kc group only needs the exps
            # emitted so far.  OT(3A) uses the ps3 slot, OT(3B) the ps1 slot.
            fb3 = [[] for _ in range(KC)]
            for th, (pool, tg) in ((0, (ps3, "ot")), (1, (ps1, "st"))):
                t3 = ot_head_thunks(3, th, (pool, tg))
                # t3 = [alloc, n0kc02, n0kc35, n0kc68, norm0,
                #       n1kc02, n1kc35, n1kc68, norm1]
                fb3[5] += [t3[0], t3[1], t3[5]]          # alloc, kc0-2 both n
                fb3[6] += [t3[2], t3[6]]                 # kc3-5 both n
                fb3[8] += [t3[3], t3[4], t3[7], t3[8]]   # kc6-8 + norms
            st_pair(3, merge(spread(ot_thunks(2), 0, 4), fb3))

            # ---------- proj + residual ----------
            outsb = sb.tile([128, 4, HW], F32, tag="outsb")
            proj_pools = [(ps3, "ot"), (ps1, "st"), (psA, "stA"), (psB, "stB")]
            for m in range(4):
                pool_m, tag_m = proj_pools[m]
                ps = pool_m.tile([128, HW], F32, tag=tag_m, name=f"proj{m}")
                for n in range(2):
                    for kc in range(4):
                        nc.tensor.matmul(ps[:, n*512:(n+1)*512],
                                         wp[:, kc, m*128:(m+1)*128],
                                         Ofull[:, kc, n*512:(n+1)*512],
                                         start=(kc == 0), stop=(kc == 3))
                nc.vector.scalar_tensor_tensor(outsb[:, m, :], ps[:], 1.0,
                                               x_sb[:, m, :],
                                               op0=ALU.mult, op1=ALU.add)
                nc.sync.dma_start(
                    out_d.ap().rearrange("(a p) m -> p a m", p=128)[:, m, :],
                    outsb[:, m, :])

    nc.compile()
    return nc


def _get_nc():
    global _cached_nc
    if _cached_nc is None:
        _cached_nc = _build()
    return _cached_nc


def kernel(**inputs):
    global LAST_RESULTS
    x = np.ascontiguousarray(np.asarray(inputs["x"], dtype=np.float32))
    context = np.asarray(inputs["context"], dtype=np.float32)
    mask = np.asarray(inputs["mask"])
    gn_w = np.asarray(inputs["gn_w"], dtype=np.float32)
    gn_b = np.asarray(inputs["gn_b"], dtype=np.float32)
    qkv_w = np.asarray(inputs["qkv_w"], dtype=np.float32)
    ckv_w = np.asarray(inputs["ckv_w"], dtype=np.float32)
    proj_w = np.asarray(inputs["proj_w"], dtype=np.float32)
    # biases are zero in this problem's setup; fold-capable but unused
    bf = ml_dtypes.bfloat16

    wqT = np.ascontiguousarray((qkv_w[0:DIM].T * SCALE).astype(bf))
    wkT = np.ascontiguousarray((qkv_w[DIM:2*DIM].T * SCALE).astype(bf))
    wvT = np.ascontiguousarray(qkv_w[2*DIM:3*DIM].T.astype(bf))
    wpT = np.ascontiguousarray(proj_w.T.astype(bf))
    wckT = np.ascontiguousarray((ckv_w[0:DIM].T * SCALE).astype(bf))
    wcvT = np.ascontiguousarray(ckv_w[DIM:2*DIM].T.astype(bf))
    gidx = np.arange(DIM) // (DIM // GROUPS)
    ind = (gidx[:, None] == np.arange(GROUPS)[None, :]).astype(np.float32)
    indT = np.ascontiguousarray(ind.T)

    shared = {"wqT": wqT, "wkT": wkT, "wvT": wvT, "wpT": wpT,
              "wckT": wckT, "wcvT": wcvT,
              "gnw": gn_w, "gnb": gn_b, "ind": ind, "indT": indT}
    in_maps = []
    for b in range(B):
        mbias = ((mask[b].astype(np.float32) - 1.0) * 30000.0).reshape(L, 1)
        im = dict(shared)
        im["x"] = x[b].reshape(DIM, HW)
        im["ctxT"] = np.ascontiguousarray(context[b].T.astype(bf))
        im["mbias"] = mbias
        in_maps.append(im)

    nc = _get_nc()
    res = run_bass_kernel_spmd(nc, in_maps, core_ids=list(range(NCORES)),
                               trace=PROFILE)
    LAST_RESULTS = res
    out = np.stack([res.results[b]["out"].reshape(DIM, H, W) for b in range(B)])
    return out.astype(np.float32)
